# revision 29
# baseline (speedup 1.0000x reference)
"""DeepSeek-style MoE layer (group-limited top-k routing + SwiGLU experts)
as a Bass/Tile kernel for 8 Trainium2 NeuronCores.

Sharding: expert-parallel. Core c owns routed experts {2c, 2c+1} and a
1/8 slice (along inter dim) of the shared MLP. Every core redundantly
computes the (tiny) router over all tokens, densely evaluates its two
experts over all tokens weighted by its combine columns, and emits a
partial [D, T] output. Host sums the 8 partials and transposes.

Layout: tokens on the matmul free axis throughout —
    out[d, t] = sum_i dpT[i, d] * prod[i, t],
    prod[i, t] = silu(gate[i, t]) * up[i, t] * combine[t, e]
so no on-device transposes are needed in the expert path (weights are
pre-transposed on host, which is pure layout work).

Precision: expert matmuls in bf16 (4x the fp32 PE rate on TRN2; fp32
matmuls lower to LO/HI pairs at half the streaming clock). Routing is
kept entirely in fp32: top-k decision margins on the logits are ~3.7e-5,
far above fp32 matmul noise but below bf16 noise, so bf16 routing could
flip expert selections vs the reference.

Schedule: the gate/up/product stream depends only on x(bf16) + streamed
weights, so it is emitted FIRST and runs from ~7us; routing (needing the
fp32 x load) overlaps; combine weights are applied as a late in-place
scale on the product tiles; the down-proj accumulates both experts plus
the shared slice into one PSUM bank per output tile.
"""

import ml_dtypes
import numpy as np

import concourse.bass as bass
import concourse.bacc as bacc
import concourse.mybir as mybir
import concourse.tile as tile
from concourse.bass_utils import run_bass_kernel_spmd
from concourse.masks import make_identity

T, D = 1024, 1024
E, K = 16, 4
G, TG = 4, 2
INTER = 512
SHARED_INTER = 1024
ROUTE_SCALE = 2.5

N_CORES = 8
EPC = E // N_CORES            # experts per core
SH = SHARED_INTER // N_CORES  # shared-inter slice per core

F32 = mybir.dt.float32
BF16 = mybir.dt.bfloat16
NEG = -1.0e9

P = 128          # partitions
TT = T // P      # token tiles (8)
DC = D // P      # d chunks (8)
IT = INTER // P  # inter tiles per expert (4)
TH = T // 512    # token halves (free-dim tiles of 512)


def build_nc(sim_safe=False):
    nc = bacc.Bacc()

    xTb = nc.dram_tensor("xTb", [D, T], BF16, kind="ExternalInput")
    xTrb = nc.dram_tensor("xTrb", [D, T], BF16, kind="ExternalInput")
    gwTb = nc.dram_tensor("gwTb", [D, E], BF16, kind="ExternalInput")
    gwTrb = nc.dram_tensor("gwTrb", [D, E], BF16, kind="ExternalInput")
    bias_rep = nc.dram_tensor("bias_rep", [1, P], F32, kind="ExternalInput")
    esel = nc.dram_tensor("esel", [EPC, E, P], F32, kind="ExternalInput")
    wg = nc.dram_tensor("wg", [EPC, D, INTER], BF16, kind="ExternalInput")
    wu = nc.dram_tensor("wu", [EPC, D, INTER], BF16, kind="ExternalInput")
    wd = nc.dram_tensor("wd", [EPC, INTER, D], BF16, kind="ExternalInput")
    shg = nc.dram_tensor("shg", [D, SH], BF16, kind="ExternalInput")
    shu = nc.dram_tensor("shu", [D, SH], BF16, kind="ExternalInput")
    shd = nc.dram_tensor("shd", [SH, D], BF16, kind="ExternalInput")
    out = nc.dram_tensor("out", [D, T], F32, kind="ExternalOutput")

    silu_fn = (mybir.ActivationFunctionType.Sigmoid if sim_safe
               else mybir.ActivationFunctionType.Silu)

    with tile.TileContext(nc) as tc:
        with (
            tc.tile_pool(name="consts", bufs=1) as consts,
            tc.tile_pool(name="xpool", bufs=1) as xpool,
            tc.tile_pool(name="wpool", bufs=1) as wpool,
            tc.tile_pool(name="route", bufs=1) as route,
            tc.tile_pool(name="wstream", bufs=16) as wstream,
            tc.tile_pool(name="prodp", bufs=1) as prodp,
            tc.tile_pool(name="gu_sb", bufs=3) as gu_sb,
            tc.tile_pool(name="outsb", bufs=3) as outsb,
            tc.tile_pool(name="ps_misc", bufs=2, space="PSUM") as ps_misc,
            tc.tile_pool(name="ps_gu", bufs=2, space="PSUM") as ps_gu,
            tc.tile_pool(name="ps_out", bufs=2, space="PSUM") as ps_out,
        ):
            # ---------- constants ----------
            ident = consts.tile([P, P], F32)
            make_identity(nc, ident)
            ones_row = consts.tile([1, P], F32)
            nc.vector.memset(ones_row, 1.0)
            ones_sq = consts.tile([P, P], F32)
            nc.vector.memset(ones_sq, 1.0)
            ones_b = consts.tile([P, P], BF16)
            nc.vector.memset(ones_b, 1.0)
            ident_b = consts.tile([P, P], BF16)
            nc.vector.tensor_copy(ident_b, ident)

            # ---------- PE clock warmup ----------
            # The PE idles ~14us for the first input DMA and starts
            # HAM-throttled (1.2GHz); the first ~12 real matmuls would run
            # at 427ns instead of 216ns. Fill the idle window with one long
            # bf16 ACCUMULATION group (back-to-back streaming, like a real
            # chunk loop — rotating start=True slots serialize on PSUM WAW
            # drains and never warm the clock). Ends ~13.5us, right before
            # the first activations land.
            warm_w = consts.tile([P, P], BF16)
            nc.vector.memset(warm_w, 0.0)
            warm_x = consts.tile([P, 512], BF16)
            nc.vector.memset(warm_x, 0.0)
            warm_ps = ps_misc.tile([P, 512], F32, tag="misc", name="warm_ps")
            N_WARM = 20
            for w in range(N_WARM):
                nc.tensor.matmul(warm_ps, warm_w, warm_x,
                                 start=(w == 0), stop=(w == N_WARM - 1))
            warm_out = consts.tile([1, 1], F32)
            nc.vector.tensor_copy(warm_out, warm_ps[:1, :1])

            # ---------- loads, latency-critical first ----------
            # first weight chunk + bf16 activations unblock the PE stream
            wgs, wus = [], []

            def load_guw(j, it):
                wgv = wg[j].rearrange("(c p) i -> p c i", p=P)
                wuv = wu[j].rearrange("(c p) i -> p c i", p=P)
                its = slice(it * P, (it + 1) * P)
                wgch = wstream.tile([P, DC, P], BF16, name=f"wgch{j}_{it}",
                                    tag="wgch")
                nc.sync.dma_start(out=wgch, in_=wgv[:, :, its])
                wuch = wstream.tile([P, DC, P], BF16, name=f"wuch{j}_{it}",
                                    tag="wuch")
                nc.sync.dma_start(out=wuch, in_=wuv[:, :, its])
                wgs.append(wgch)
                wus.append(wuch)

            xtbv = xTb.rearrange("(c p) t -> p c t", p=P)
            xtb0 = xpool.tile([P, DC, 512], BF16)
            nc.sync.dma_start(out=xtb0, in_=xtbv[:, :, :512])
            load_guw(0, 0)
            xtb1 = xpool.tile([P, DC, 512], BF16)
            nc.sync.dma_start(out=xtb1, in_=xtbv[:, :, 512:])
            xtbs = [xtb0, xtb1]
            for it in range(1, IT):
                load_guw(0, it)
            # routing terms (bf16 x/gate-weight + bf16 residuals)
            xrb = xpool.tile([P, DC, T], BF16)
            nc.sync.dma_start(out=xrb, in_=xTrb.rearrange("(c p) t -> p c t", p=P))
            gwb_sb = consts.tile([P, DC, E], BF16)
            nc.sync.dma_start(out=gwb_sb, in_=gwTb.rearrange("(c p) e -> p c e", p=P))
            gwrb_sb = consts.tile([P, DC, E], BF16)
            nc.sync.dma_start(out=gwrb_sb,
                              in_=gwTrb.rearrange("(c p) e -> p c e", p=P))
            for it in range(IT):
                load_guw(1, it)
            bias_sb = consts.tile([1, P], F32)
            nc.sync.dma_start(out=bias_sb, in_=bias_rep[:, :])
            esel_sb = consts.tile([E, EPC, P], F32)
            nc.sync.dma_start(out=esel_sb, in_=esel.rearrange("j e p -> e j p"))
            # down + shared weights (needed later)
            shg_sb = wpool.tile([P, DC, SH], BF16)
            shu_sb = wpool.tile([P, DC, SH], BF16)
            nc.sync.dma_start(out=shg_sb, in_=shg.rearrange("(c p) i -> p c i", p=P))
            nc.sync.dma_start(out=shu_sb, in_=shu.rearrange("(c p) i -> p c i", p=P))
            wd_sb = [wpool.tile([P, IT, D], BF16, name=f"wd_sb{j}", tag=f"wd{j}")
                     for j in range(EPC)]
            for j in range(EPC):
                nc.sync.dma_start(out=wd_sb[j],
                                  in_=wd[j].rearrange("(c p) d -> p c d", p=P))
            shd_sb = wpool.tile([P, D], BF16)
            nc.sync.dma_start(out=shd_sb, in_=shd[:, :])

            # ---------- expert gate/up + unweighted products ----------
            # Independent of routing. Emitted around the routing PE work so
            # the routing DVE chain overlaps expert matmuls.
            prods = [prodp.tile([P, IT, T], BF16, name=f"prod{j}", tag=f"prod{j}")
                     for j in range(EPC)]

            def gate_up(j, its_list=None):
                prod = prods[j]
                for it in (its_list if its_list is not None else range(IT)):
                    wgch, wuch = wgs[j * IT + it], wus[j * IT + it]
                    for th in range(TH):
                        ts512 = slice(th * 512, (th + 1) * 512)
                        pg = ps_gu.tile([P, 512], F32, name="pg", tag="pg")
                        for c in range(DC):
                            nc.tensor.matmul(pg, wgch[:, c, :], xtbs[th][:, c, :],
                                             start=(c == 0), stop=(c == DC - 1))
                        pu = ps_gu.tile([P, 512], F32, name="pu", tag="pu")
                        for c in range(DC):
                            nc.tensor.matmul(pu, wuch[:, c, :], xtbs[th][:, c, :],
                                             start=(c == 0), stop=(c == DC - 1))
                        sg = gu_sb.tile([P, 512], F32, name="sg", tag="sg")
                        nc.scalar.activation(sg, pg, silu_fn)
                        if sim_safe:
                            sg2 = gu_sb.tile([P, 512], F32, name="sg2", tag="sg2")
                            nc.vector.tensor_tensor(sg2, pg, sg,
                                                    op=mybir.AluOpType.mult)
                            sg = sg2
                        nc.vector.tensor_tensor(prod[:, it, ts512], pu, sg,
                                                op=mybir.AluOpType.mult)

            def shared_gate_up():
                for th in range(TH):
                    ts512 = slice(th * 512, (th + 1) * 512)
                    pg = ps_gu.tile([P, 512], F32, name="pg", tag="pg")
                    for c in range(DC):
                        nc.tensor.matmul(pg, shg_sb[:, c, :], xtbs[th][:, c, :],
                                         start=(c == 0), stop=(c == DC - 1))
                    pu = ps_gu.tile([P, 512], F32, name="pu", tag="pu")
                    for c in range(DC):
                        nc.tensor.matmul(pu, shu_sb[:, c, :], xtbs[th][:, c, :],
                                         start=(c == 0), stop=(c == DC - 1))
                    sg = gu_sb.tile([P, 512], F32, name="sg", tag="sg")
                    nc.scalar.activation(sg, pg, silu_fn)
                    if sim_safe:
                        sg2 = gu_sb.tile([P, 512], F32, name="sg2", tag="sg2")
                        nc.vector.tensor_tensor(sg2, pg, sg,
                                                op=mybir.AluOpType.mult)
                        sg = sg2
                    nc.vector.tensor_tensor(shprod[:, ts512], pu, sg,
                                            op=mybir.AluOpType.mult)

            shprod = prodp.tile([P, T], BF16, name="shprod", tag="shprod")

            gate_up(0)

            # ---------- routing (fp32, overlaps the stream above) ----------
            # ---------- routing (fp32, overlaps the stream above) ----------
            # z = xb@gwb + xr@gwb + xb@gwr  (bf16 3-term; dropped xr@gwr and
            # double-residual terms are ~1e-6 on the scores, 30x under the
            # 3.7e-5 top-k decision margin of this input)
            scT = route.tile([32, T], F32)
            nc.vector.memset(scT, 0.0)
            for th in range(TH):
                zt = ps_misc.tile([E, 512], F32, tag="misc", name="zt")
                k, last = 0, 3 * DC - 1
                xr_th = xrb[:, :, th * 512:(th + 1) * 512]
                for w_sb, rhs in ((gwb_sb, None), (gwb_sb, xr_th), (gwrb_sb, None)):
                    for c in range(DC):
                        r = xtbs[th][:, c, :] if rhs is None else rhs[:, c, :]
                        nc.tensor.matmul(zt, w_sb[:, c, :], r,
                                         start=(k == 0), stop=(k == last))
                        k += 1
                nc.scalar.activation(scT[:E, th * 512:(th + 1) * 512], zt,
                                     mybir.ActivationFunctionType.Sigmoid)

            # scores [t, e] via DVE 32x32 block transposes (keeps PE free);
            # columns 16:32 of each block are padding, never read.
            scores32 = route.tile([P, TT, 32], F32, name="scores32")
            for tt in range(TT):
                for b in range(4):
                    nc.vector.transpose(
                        scores32[32 * b:32 * (b + 1), tt, :],
                        scT[:, tt * P + 32 * b:tt * P + 32 * b + 32])
            scores = scores32[:, :, :E]  # strided view, [128, TT, E]

            # biased scores s = scores + bias (bias pattern repeats per tile)
            bias_bc = ps_misc.tile([P, P], F32, tag="misc", name="bias_bc")
            nc.tensor.matmul(bias_bc, ones_row, bias_sb, start=True, stop=True)
            s_all = route.tile([P, TT, E], F32)
            nc.vector.tensor_tensor(
                s_all, scores,
                bias_bc.rearrange("p (a b) -> p a b", b=E),
                op=mybir.AluOpType.add)

            grp = s_all.rearrange("p t (g r) -> p (t g) r", r=E // G)  # [128,32,4]

            def bcast_last(ap2d, n):
                # [p, m] -> [p, m, n] with stride-0 inner axis
                a = ap2d.ap
                return bass.AP(tensor=ap2d.tensor, offset=ap2d.offset,
                               ap=list(a) + [[0, n]])

            # group score = top2-sum within each group of 4
            m1 = route.tile([P, TT * G], F32)
            nc.vector.tensor_reduce(m1, grp, axis=mybir.AxisListType.X,
                                    op=mybir.AluOpType.max)
            eq = route.tile([P, TT * G, E // G], F32)
            nc.vector.tensor_tensor(eq, grp, bcast_last(m1, E // G),
                                    op=mybir.AluOpType.is_equal)
            nc.vector.tensor_scalar_mul(eq, eq, NEG)
            s2 = route.tile([P, TT * G, E // G], F32)
            nc.vector.tensor_tensor(s2, grp, eq, op=mybir.AluOpType.add)
            m2 = route.tile([P, TT * G], F32)
            nc.vector.tensor_reduce(m2, s2, axis=mybir.AxisListType.X,
                                    op=mybir.AluOpType.max)
            gsc = route.tile([P, TT * G], F32)
            nc.vector.tensor_tensor(gsc, m1, m2, op=mybir.AluOpType.add)

            # top-2 groups per token: threshold = 2nd-largest group score
            gv = gsc.rearrange("p (t g) -> p t g", g=G)  # [128, 8, 4]
            gm1 = route.tile([P, TT], F32)
            nc.vector.tensor_reduce(gm1, gv, axis=mybir.AxisListType.X,
                                    op=mybir.AluOpType.max)
            geq = route.tile([P, TT, G], F32)
            nc.vector.tensor_tensor(geq, gv, bcast_last(gm1, G),
                                    op=mybir.AluOpType.is_equal)
            nc.vector.tensor_scalar_mul(geq, geq, NEG)
            gs2 = route.tile([P, TT, G], F32)
            nc.vector.tensor_tensor(gs2, gv, geq, op=mybir.AluOpType.add)
            gm2 = route.tile([P, TT], F32)
            nc.vector.tensor_reduce(gm2, gs2, axis=mybir.AxisListType.X,
                                    op=mybir.AluOpType.max)
            gmask = route.tile([P, TT, G], F32)
            nc.vector.tensor_tensor(gmask, gv, bcast_last(gm2, G),
                                    op=mybir.AluOpType.is_ge)

            # masked scores; expand gmask over the 4 experts of each group
            gmask_x = bass.AP(
                tensor=gmask.tensor, offset=gmask.offset,
                ap=list(gmask.ap) + [[0, E // G]])       # [128, 8, 4, 4]
            sm = route.tile([P, TT, G, E // G], F32)
            nc.vector.tensor_tensor(
                sm, s_all.rearrange("p t (g r) -> p t g r", r=E // G),
                gmask_x, op=mybir.AluOpType.mult)

            # per-token top-4 threshold via max8 (4th largest of 16)
            tau8 = route.tile([P, TT, 8], F32)
            smf = sm.rearrange("p t g r -> p t (g r)")   # [128, 8, 16]
            for tt in range(TT):
                nc.vector.max(tau8[:, tt, :], smf[:, tt, :])
            tau = bass.AP(tensor=tau8.tensor, offset=tau8.offset + 3,
                          ap=[tau8.ap[0], [8, TT], [0, E]])  # [128, 8, 16] bcast
            sel = route.tile([P, TT, E], F32)
            nc.vector.tensor_tensor(sel, smf, tau, op=mybir.AluOpType.is_ge)

            # combine weights: normalized original scores at selected slots
            wsel = route.tile([P, TT, E], F32)
            nc.vector.tensor_tensor(wsel, scores, sel, op=mybir.AluOpType.mult)
            den = route.tile([P, TT], F32)
            nc.vector.tensor_reduce(den, wsel, axis=mybir.AxisListType.X,
                                    op=mybir.AluOpType.add)
            rec = route.tile([P, TT], F32)
            nc.vector.reciprocal(rec, den)
            nc.vector.tensor_scalar_mul(rec, rec, ROUTE_SCALE)
            comb = route.tile([P, TT, E], F32)
            nc.vector.tensor_tensor(comb, wsel, bcast_last(rec, E),
                                    op=mybir.AluOpType.mult)

            gate_up(1, [0, 1])

            # per-local-expert combine row broadcast across partitions,
            # without transposes: masked[k, tt, ti] = comb[k, tt, e_j] *
            # ident[k, ti]; summing over k (ones matmul) leaves
            # W[p, tt*128+ti] = comb[ti, tt, e_j] on every partition p.
            Wsb = [route.tile([P, T], F32, name=f"Wsb{j}", tag=f"wrow{j}")
                   for j in range(EPC)]
            ident_x = bass.AP(tensor=ident.tensor, offset=ident.offset,
                              ap=[ident.ap[0], [0, TT], [1, P]])
            for j in range(EPC):
                e_j = None  # local expert j selects column via esel on host;
                # here the expert index is j's column in comb, resolved by
                # esel_sb at matmul time in the previous design.  With the
                # masked-identity trick we instead need comb[:, :, e] where
                # e = core's j-th expert — but e differs per core while the
                # program is SPMD.  Recover SPMD-uniformity by first
                # collapsing comb over experts with the per-core one-hot:
                # cj[k, tt] = sum_e comb[k, tt, e] * esel_onehot[e].
                pass
            # cj via DVE: comb [128, TT, E] * esel row bcast, reduced over E
            cj = [route.tile([P, TT], F32, name=f"cj{j}", tag=f"cj{j}")
                  for j in range(EPC)]
            esel_f = consts.tile([P, EPC, E], F32)
            nc.sync.dma_start(
                out=esel_f,
                in_=bass.AP(tensor=esel, offset=0,
                            ap=[[0, P], [E * P, EPC], [P, E]]))
            for j in range(EPC):
                er = bass.AP(tensor=esel_f.tensor,
                             offset=esel_f.offset + j * E,
                             ap=[esel_f.ap[0], [0, TT], [1, E]])
                cjt = route.tile([P, TT, E], F32, name=f"cjt{j}", tag="cjt")
                nc.vector.tensor_tensor(cjt, comb, er, op=mybir.AluOpType.mult)
                nc.vector.tensor_reduce(cj[j], cjt, axis=mybir.AxisListType.X,
                                        op=mybir.AluOpType.add)
            # bf16 Wb matmuls via value+residual split of cj (the one-hot
            # masked sum makes each term exact in fp32 PSUM; combined error
            # ~2^-17, vs 2x slower fp32 2-pass matmuls)
            identb_x = bass.AP(tensor=ident_b.tensor, offset=ident_b.offset,
                               ap=[ident_b.ap[0], [0, TT], [1, P]])
            for j in range(EPC):
                cjb = route.tile([P, TT], BF16, name=f"cjb{j}", tag="cjb")
                nc.vector.tensor_copy(cjb, cj[j])
                cjr = route.tile([P, TT], BF16, name=f"cjr{j}", tag="cjr")
                nc.vector.tensor_tensor(cjr, cj[j], cjb,
                                        op=mybir.AluOpType.subtract)
                mb = route.tile([P, TT, P], BF16, name=f"mb{j}", tag="mb")
                nc.vector.tensor_tensor(mb, bcast_last(cjb, P), identb_x,
                                        op=mybir.AluOpType.mult)
                mr = route.tile([P, TT, P], BF16, name=f"mr{j}", tag="mr")
                nc.vector.tensor_tensor(mr, bcast_last(cjr, P), identb_x,
                                        op=mybir.AluOpType.mult)
                for th in range(TH):
                    ts4 = slice(th * (TT // 2), (th + 1) * (TT // 2))
                    wb = ps_misc.tile([P, 512], F32, tag="misc", name="wb")
                    nc.tensor.matmul(wb, ones_b, mb[:, ts4, :],
                                     start=True, stop=False)
                    nc.tensor.matmul(wb, ones_b, mr[:, ts4, :],
                                     start=False, stop=True)
                    nc.scalar.activation(Wsb[j][:, th * 512:(th + 1) * 512], wb,
                                         mybir.ActivationFunctionType.Copy)

            gate_up(1, [2, 3])
            shared_gate_up()

            # ---------- apply combine weights in place on the products ----------
            for th in range(TH):
                for j in range(EPC):
                    ts512 = slice(th * 512, (th + 1) * 512)
                    w_b = bass.AP(tensor=Wsb[j].tensor,
                                  offset=Wsb[j].offset + th * 512,
                                  ap=[Wsb[j].ap[0], [0, IT], [1, 512]])
                    nc.vector.tensor_tensor(prods[j][:, :, ts512],
                                            prods[j][:, :, ts512], w_b,
                                            op=mybir.AluOpType.mult)

            # ---------- down-proj: both experts + shared into one bank ----------
            for th in range(TH):
                ts512 = slice(th * 512, (th + 1) * 512)
                for dt in range(DC):
                    po = ps_out.tile([P, 512], F32, name="po", tag="po")
                    n_mm = EPC * IT + 1
                    k = 0
                    for j in range(EPC):
                        for ic in range(IT):
                            nc.tensor.matmul(
                                po, wd_sb[j][:, ic, dt * P:(dt + 1) * P],
                                prods[j][:, ic, ts512],
                                start=(k == 0), stop=(k == n_mm - 1))
                            k += 1
                    nc.tensor.matmul(po, shd_sb[:, dt * P:(dt + 1) * P],
                                     shprod[:, ts512], start=False, stop=True)
                    ob = outsb.tile([P, 512], F32, name="ob", tag="ob")
                    nc.scalar.activation(ob, po,
                                         mybir.ActivationFunctionType.Copy)
                    nc.sync.dma_start(out=out[dt * P:(dt + 1) * P, ts512], in_=ob)

    nc.compile()
    return nc


_NC_CACHE = {}


def _get_nc():
    if "nc" not in _NC_CACHE:
        _NC_CACHE["nc"] = build_nc()
    return _NC_CACHE["nc"]


def make_in_maps(inputs):
    f = lambda a: np.ascontiguousarray(np.asarray(a), dtype=np.float32)
    x = f(inputs["x"])
    gate_w = f(inputs["gate_w"])
    gate_bias = f(inputs["gate_bias"])
    gate_projs = f(inputs["gate_projs"])
    up_projs = f(inputs["up_projs"])
    down_projs = f(inputs["down_projs"])
    shared_gate = f(inputs["shared_gate"])
    shared_up = f(inputs["shared_up"])
    shared_down = f(inputs["shared_down"])

    xT = np.ascontiguousarray(x.T)
    xTb = xT.astype(ml_dtypes.bfloat16)
    xTrb = (xT - xTb.astype(np.float32)).astype(ml_dtypes.bfloat16)
    gwT = np.ascontiguousarray(gate_w.T)
    gwTb = gwT.astype(ml_dtypes.bfloat16)
    gwTrb = (gwT - gwTb.astype(np.float32)).astype(ml_dtypes.bfloat16)
    bias_rep = np.ascontiguousarray(np.tile(gate_bias, TT)[None, :])
    shgT = np.ascontiguousarray(shared_gate.T)   # [D, SHARED_INTER]
    shuT = np.ascontiguousarray(shared_up.T)
    shdT = np.ascontiguousarray(shared_down.T)   # [SHARED_INTER, D]

    in_maps = []
    for c in range(N_CORES):
        es = np.zeros((EPC, E, P), np.float32)
        for j in range(EPC):
            es[j, EPC * c + j, :] = 1.0
        in_maps.append({
            "xTb": xTb,
            "xTrb": xTrb,
            "gwTb": gwTb,
            "gwTrb": gwTrb,
            "bias_rep": bias_rep,
            "esel": es,
            "wg": np.ascontiguousarray(
                np.stack([gate_projs[EPC * c + j].T for j in range(EPC)])
            ).astype(ml_dtypes.bfloat16),
            "wu": np.ascontiguousarray(
                np.stack([up_projs[EPC * c + j].T for j in range(EPC)])
            ).astype(ml_dtypes.bfloat16),
            "wd": np.ascontiguousarray(
                np.stack([down_projs[EPC * c + j].T for j in range(EPC)])
            ).astype(ml_dtypes.bfloat16),
            "shg": np.ascontiguousarray(
                shgT[:, c * SH:(c + 1) * SH]).astype(ml_dtypes.bfloat16),
            "shu": np.ascontiguousarray(
                shuT[:, c * SH:(c + 1) * SH]).astype(ml_dtypes.bfloat16),
            "shd": np.ascontiguousarray(
                shdT[c * SH:(c + 1) * SH, :]).astype(ml_dtypes.bfloat16),
        })
    return in_maps


def combine_results(results):
    total = np.zeros((D, T), np.float32)
    for r in results:
        total += r["out"]
    return np.ascontiguousarray(total.T)


def kernel(**inputs):
    in_maps = make_in_maps(inputs)
    nc = _get_nc()
    res = run_bass_kernel_spmd(nc, in_maps, list(range(N_CORES)))
    return combine_results(res.results)
